# revision 14
# baseline (speedup 1.0000x reference)
"""Trainium2 Bass kernel for nn_Block_36438502540029 (involution CNN block).

Structure per core (data-parallel over batch, 2 images/core):
  conv1 (1x1, 512->128) -> ReLU -> padded bf16 buffer
  reduce (1x1, 128->32, M-replicated x4) -> ReLU -> w1rep
  involution apply, processed per image (half-spatial) so the out2
    accumulator needs only 2 PSUM banks, leaving 3 ping-pong slots for
    the per-tap broadcast PSUM tiles:
      span+broadcast: per-tap stationary Ws_dup[32,128] (rows duplicated
        16x host-side), row-tiled pair matmuls -> kernel maps in PSUM
      drained pairs: ACT drains PSUM->SBUF bf16; DVE bf16 muls vs
        shifted padded x1
      direct pairs: DVE muls read the PSUM maps directly (fp32, 1x)
      PE identity-matmul accumulation into the half-image out2 PSUM
  conv3 (1x1, 128->512) + identity residual via extra matmul + bias

All matmuls bf16 (full rate); PSUM accumulation fp32. Dependency-free
filler matmuls into an unused PSUM strip keep the PE HAM clock-gate warm.
"""
import numpy as np
import ml_dtypes
from contextlib import ExitStack

import concourse.bass as bass
import concourse.tile as tile
from concourse import bacc, mybir
from concourse import bass_utils

bf16 = mybir.dt.bfloat16
f32 = mybir.dt.float32
AF = mybir.ActivationFunctionType
ALU = mybir.AluOpType
BF = ml_dtypes.bfloat16

N_CORES = 8
B, CIN, H, W = 16, 512, 28, 28
BL = B // N_CORES            # images per core
CMID, CRED, G, GCH = 128, 32, 8, 16
KS, PD = 7, 3                # kernel size, pad
HWPX = H * W                 # 784
NPX = BL * HWPX              # 1568
PW = W + 2 * PD              # 34
PIMG = PW * PW               # 1156
NPAD = BL * PIMG + 8         # 2320 (slack for strided quarter views)
NTAP = KS * KS               # 49
NPAIR = (NTAP + 1) // 2      # 25 (last pair single)

CHUNKS = [(0, 512), (512, 512), (1024, 512), (1536, 32)]      # ragged 1568
HCHUNKS = [(0, 512), (512, 272)]                              # ragged 784
QW = 392                     # quarter width (half of one image)

# pairs whose taps use the direct-from-PSUM DVE path (no ACT drain)
DIRECT_PAIRS = frozenset({4, 9, 14, 19})

_prog_cache = {}


def _build_program(direct_pairs=DIRECT_PAIRS, use_stt=False):
    nc = bacc.Bacc("TRN2", num_devices=N_CORES, debug=False)

    dr = {}
    dr["x"] = nc.dram_tensor("x", [128, 4 * NPX], bf16, kind="ExternalInput")
    dr["w1t"] = nc.dram_tensor("w1t", [128, 512], bf16, kind="ExternalInput")
    dr["wrt"] = nc.dram_tensor("wrt", [128, 128], bf16, kind="ExternalInput")
    dr["wsd"] = nc.dram_tensor("wsd", [64, NPAIR * 128], bf16, kind="ExternalInput")
    dr["w3t"] = nc.dram_tensor("w3t", [128, 512], bf16, kind="ExternalInput")
    dr["ident"] = nc.dram_tensor("ident", [128, 128], bf16, kind="ExternalInput")
    dr["b1"] = nc.dram_tensor("b1", [128, 1], f32, kind="ExternalInput")
    dr["brr"] = nc.dram_tensor("brr", [128, 1], f32, kind="ExternalInput")
    dr["bsd"] = nc.dram_tensor("bsd", [128, NTAP], f32, kind="ExternalInput")
    dr["b3"] = nc.dram_tensor("b3", [128, 4], f32, kind="ExternalInput")
    y = nc.dram_tensor("y", [128, 4 * NPX], bf16, kind="ExternalOutput")

    with tile.TileContext(nc) as tc:
        with ExitStack() as ctx:
            const = ctx.enter_context(tc.tile_pool(name="const", bufs=1))
            sbuf = ctx.enter_context(tc.tile_pool(name="sbuf", bufs=1))
            wsmp = ctx.enter_context(tc.tile_pool(name="wsm", bufs=6))
            prodp = ctx.enter_context(tc.tile_pool(name="prod", bufs=10))
            ystg = ctx.enter_context(tc.tile_pool(name="ystg", bufs=2))
            pso = ctx.enter_context(tc.tile_pool(name="pso", bufs=1, space="PSUM"))
            psB = ctx.enter_context(tc.tile_pool(name="psB", bufs=3, space="PSUM"))

            # ---- loads: first x tile first so conv1 starts ASAP ----
            xsb = sbuf.tile([128, 4 * NPX], bf16, name="xsb")
            nc.sync.dma_start(xsb[:, 0:NPX], dr["x"].ap()[:, 0:NPX])
            w1t_sb = const.tile([128, 512], bf16, name="w1t_sb")
            nc.sync.dma_start(w1t_sb[:], dr["w1t"].ap())
            wrt_sb = const.tile([128, 128], bf16, name="wrt_sb")
            nc.sync.dma_start(wrt_sb[:], dr["wrt"].ap())
            wsd_sb = const.tile([128, NPAIR * 128], bf16, name="wsd_sb")
            nc.sync.dma_start(wsd_sb[0:64, :], dr["wsd"].ap())
            w3t_sb = const.tile([128, 512], bf16, name="w3t_sb")
            nc.sync.dma_start(w3t_sb[:], dr["w3t"].ap())
            id_sb = const.tile([128, 128], bf16, name="id_sb")
            nc.sync.dma_start(id_sb[:], dr["ident"].ap())
            b1_sb = const.tile([128, 1], f32, name="b1_sb")
            nc.sync.dma_start(b1_sb[:], dr["b1"].ap())
            brr_sb = const.tile([128, 1], f32, name="brr_sb")
            nc.sync.dma_start(brr_sb[:], dr["brr"].ap())
            bsd_sb = const.tile([128, NTAP], f32, name="bsd_sb")
            nc.sync.dma_start(bsd_sb[:], dr["bsd"].ap())
            b3_sb = const.tile([128, 4], f32, name="b3_sb")
            nc.sync.dma_start(b3_sb[:], dr["b3"].ap())
            for k in range(1, 4):
                nc.sync.dma_start(xsb[:, NPX * k:NPX * (k + 1)],
                                  dr["x"].ap()[:, NPX * k:NPX * (k + 1)])

            pad_t = sbuf.tile([128, NPAD], bf16, name="pad_t")
            nc.vector.memset(pad_t[:], 0.0)
            pad4 = pad_t[:, 0:BL * PIMG].rearrange(
                "p (b i j) -> p b i j", b=BL, i=PW, j=PW)

            # ---- conv1: out1 = relu(W1' @ x + b1); quarter-serial (392 px =
            # 14 rows, row-aligned for the strided pad write) through the
            # shared psum pool ----
            for q in range(4):
                b_, hh = q // 2, q % 2
                cps = psB.tile([128, 1024], f32, tag="bc", name=f"c1ps{q}")
                for k in range(4):
                    nc.tensor.matmul(
                        cps[:, 0:QW],
                        w1t_sb[:, 128 * k:128 * (k + 1)],
                        xsb[:, NPX * k + QW * q:NPX * k + QW * (q + 1)],
                        start=(k == 0), stop=(k == 3),
                    )
                nc.scalar.activation(
                    pad4[:, b_:b_ + 1, PD + 14 * hh:PD + 14 * hh + 14, PD:PD + W],
                    cps[:, 0:QW].rearrange("p (a i j) -> p a i j",
                                           a=1, i=14, j=W),
                    AF.Relu, bias=b1_sb[:], scale=1.0,
                )

            # ---- reduce: w1rep = relu(Wr'_rep @ out1 + br_rep), per quarter ----
            w1rep = sbuf.tile([128, NPX], bf16, name="w1rep")
            for q in range(4):
                b_, hh = q // 2, q % 2
                rps = psB.tile([128, 1024], f32, tag="bc", name=f"redps{q}")
                rhs = pad4[:, b_:b_ + 1, PD + 14 * hh:PD + 14 * hh + 14, PD:PD + W]
                nc.tensor.matmul(rps[:, 0:QW], wrt_sb[:], rhs,
                                 start=True, stop=True)
                nc.scalar.activation(
                    w1rep[:, QW * q:QW * (q + 1)], rps[:, 0:QW],
                    AF.Relu, bias=brr_sb[:], scale=1.0,
                )

            out2sb = sbuf.tile([128, NPX], bf16, name="out2sb")

            def pad_shift_half(t, himg, squeeze_q=None):
                di, dj = t // KS - PD, t % KS - PD
                if squeeze_q is None:
                    return pad4[:, himg:himg + 1,
                                PD + di:PD + di + H, PD + dj:PD + dj + W]
                hh = squeeze_q
                r0 = PD + di + 14 * hh
                off = himg * PIMG + r0 * PW + PD + dj
                return pad_t[:, off:off + 14 * PW].rearrange(
                    "p (i j) -> p i j", i=14, j=PW)[:, :, 0:W]

            # ---- involution apply, one image (half) at a time ----
            for himg in range(BL):
                o2 = pso.tile([128, 1024], f32, tag="o2", name=f"o2_{himg}")

                filler_state = {"first": True}

                def pe_filler(n=1):
                    # dependency-free matmuls into the unused strip of the
                    # out2 psum bank; they run whenever the PE would
                    # otherwise stall, keeping the HAM clock-gate warm.
                    # First one per half starts the region (defined values);
                    # it precedes the first real chunk-1 accum, whose
                    # start=True bank-clear wipes the junk harmlessly.
                    for _ in range(n):
                        nc.tensor.matmul(o2[:, 800:1024], id_sb[:],
                                         xsb[:, 0:224],
                                         start=filler_state["first"], stop=True,
                                         skip_group_check=True)
                        filler_state["first"] = False

                accum_fifo = []

                def push_accum(prods_taps):
                    for pr, t in prods_taps:
                        for ci in range(len(HCHUNKS)):
                            accum_fifo.append((pr, t, ci))

                def emit_accum(n):
                    take = accum_fifo[:n] if n else list(accum_fifo)
                    del accum_fifo[:len(take)]
                    for pr, t, ci in take:
                        off, wd = HCHUNKS[ci]
                        nc.tensor.matmul(
                            o2[:, off:off + wd], id_sb[:],
                            pr[:, off:off + wd],
                            start=(t == 0), stop=(t == NTAP - 1),
                            skip_group_check=True,
                        )

                for p in range(NPAIR):
                    taps = [t for t in (2 * p, 2 * p + 1) if t < NTAP]
                    ns = len(taps)
                    drained = p not in direct_pairs
                    prods = []
                    for s, t in enumerate(taps):
                        pr = prodp.tile([128, HWPX], bf16, tag="prod",
                                        name=f"prod{himg}_{t}")
                        prods.append(pr)
                    wm = None
                    if drained:
                        wm = wsmp.tile([128, ns * HWPX], bf16, tag="wm",
                                       name=f"wm{himg}_{p}")
                    for qq in range(2):
                        q = 2 * himg + qq
                        if qq == 0:
                            pe_filler(1)
                        bq = psB.tile([128, 1024], f32, tag="bc",
                                      name=f"bc{himg}_{p}_{qq}")
                        for s, t in enumerate(taps):
                            nc.tensor.matmul(
                                bq[:, 512 * s:512 * s + QW],
                                wsd_sb[32 * s:32 * (s + 1), 128 * p:128 * (p + 1)],
                                w1rep[32 * s:32 * (s + 1), QW * q:QW * (q + 1)],
                                start=True, stop=True,
                                tile_position=(32 * s, 0),
                            )
                        if p >= 2:
                            emit_accum(2)
                        if drained:
                            nc.scalar.activation(
                                wm[:].rearrange("p (s n) -> p s n",
                                                s=ns, n=HWPX)[
                                    :, :, QW * qq:QW * (qq + 1)],
                                bq[:].rearrange("p (s n) -> p s n",
                                                s=2, n=512)[:, 0:ns, 0:QW],
                                AF.Identity, bias=0.0, scale=1.0,
                            )
                        else:
                            for s, t in enumerate(taps):
                                if use_stt:
                                    nc.vector.scalar_tensor_tensor(
                                        prods[s][:, QW * qq:QW * (qq + 1)]
                                        .rearrange("p (i j) -> p i j",
                                                   i=14, j=W),
                                        bq[:, 512 * s:512 * s + QW].rearrange(
                                            "p (i j) -> p i j", i=14, j=W),
                                        bsd_sb[:, t:t + 1],
                                        pad_shift_half(t, himg, squeeze_q=qq),
                                        ALU.add, ALU.mult,
                                    )
                                else:
                                    nc.vector.tensor_mul(
                                        prods[s][:, QW * qq:QW * (qq + 1)]
                                        .rearrange("p (i j) -> p i j",
                                                   i=14, j=W),
                                        bq[:, 512 * s:512 * s + QW].rearrange(
                                            "p (i j) -> p i j", i=14, j=W),
                                        pad_shift_half(t, himg, squeeze_q=qq),
                                    )
                    if drained:
                        for s, t in enumerate(taps):
                            nc.vector.tensor_mul(
                                prods[s][:].rearrange("p (i j) -> p i j",
                                                      i=H, j=W),
                                wm[:, HWPX * s:HWPX * (s + 1)].rearrange(
                                    "p (i j) -> p i j", i=H, j=W),
                                pad_shift_half(t, himg),
                            )
                    push_accum([(prods[s], t) for s, t in enumerate(taps)])
                emit_accum(0)

                nc.vector.tensor_copy(
                    out2sb[:, HWPX * himg:HWPX * (himg + 1)], o2[:, 0:HWPX])

            # ---- conv3 + residual: y_m = W3'_m @ out2 + x_m + b3_m ----
            for m in range(4):
                for hh in range(2):
                    c3 = psB.tile([128, 1024], f32, tag="bc", name=f"c3_{m}_{hh}")
                    hoff = HWPX * hh
                    for (off, wd) in HCHUNKS:
                        nc.tensor.matmul(
                            c3[:, off:off + wd], w3t_sb[:, 128 * m:128 * (m + 1)],
                            out2sb[:, hoff + off:hoff + off + wd],
                            start=True, stop=False, skip_group_check=True,
                        )
                    for (off, wd) in HCHUNKS:
                        nc.tensor.matmul(
                            c3[:, off:off + wd], id_sb[:],
                            xsb[:, NPX * m + hoff + off:NPX * m + hoff + off + wd],
                            start=False, stop=True, skip_group_check=True,
                        )
                    ysb = ystg.tile([128, HWPX], bf16, tag="y", name=f"y{m}_{hh}")
                    nc.scalar.activation(ysb[:], c3[:, 0:HWPX], AF.Identity,
                                         bias=b3_sb[:, m:m + 1], scale=1.0)
                    nc.sync.dma_start(y.ap()[:, NPX * m + hoff:NPX * m + hoff + HWPX],
                                      ysb[:])

    nc.compile()
    return nc


def get_program(all_direct=False):
    key = "nc_all_direct" if all_direct else "nc"
    if key not in _prog_cache:
        dp = frozenset(range(NPAIR)) if all_direct else DIRECT_PAIRS
        _prog_cache[key] = _build_program(dp, use_stt=all_direct)
    return _prog_cache[key]


def _host_prep(inputs):
    """Fold scales into weights; build per-core DRAM tensor layouts."""
    x = np.asarray(inputs["x"], np.float32)
    W1 = np.asarray(inputs["W1"], np.float32) * np.asarray(inputs["s1"], np.float32)[:, None]
    Wr = np.asarray(inputs["Wr"], np.float32) * np.asarray(inputs["sr"], np.float32)[:, None]
    Ws = np.asarray(inputs["Ws"], np.float32)
    W3 = np.asarray(inputs["W3"], np.float32) * np.asarray(inputs["s3"], np.float32)[:, None]
    b1 = np.asarray(inputs["b1"], np.float32)
    br = np.asarray(inputs["br"], np.float32)
    bs = np.asarray(inputs["bs"], np.float32)
    b3 = np.asarray(inputs["b3"], np.float32)

    w1t = np.ascontiguousarray(
        W1.T.reshape(4, 128, 128).transpose(1, 0, 2).reshape(128, 512)).astype(BF)
    wrt = np.tile(Wr.T, (1, 4)).astype(BF)
    wsd = np.zeros((64, NPAIR * 128), np.float32)
    WsT = Ws.reshape(G, NTAP, CRED)  # [g, t, j]
    for p in range(NPAIR):
        for s in range(2):
            t = 2 * p + s
            if t >= NTAP:
                continue
            blk = WsT[:, t, :].T  # [j, g]
            wsd[32 * s:32 * s + 32, 128 * p:128 * (p + 1)] = np.repeat(
                blk, GCH, axis=1)
    wsd = wsd.astype(BF)
    w3t = W3.T.astype(BF)
    ident = np.eye(128, dtype=np.float32).astype(BF)
    bsd = np.repeat(bs.reshape(G, NTAP), GCH, axis=0)
    bsd = np.ascontiguousarray(bsd).astype(np.float32)

    base = {
        "w1t": w1t, "wrt": wrt, "wsd": wsd, "w3t": w3t, "ident": ident,
        "b1": b1.reshape(128, 1).astype(np.float32),
        "brr": np.tile(br, 4).reshape(128, 1).astype(np.float32),
        "bsd": bsd,
        "b3": np.ascontiguousarray(b3.reshape(4, 128).T).astype(np.float32),
    }
    in_maps = []
    for c in range(N_CORES):
        xs = x[BL * c:BL * (c + 1)]
        xc = np.ascontiguousarray(
            xs.reshape(BL, 4, 128, HWPX).transpose(2, 1, 0, 3).reshape(128, 4 * NPX)
        ).astype(BF)
        m = dict(base)
        m["x"] = xc
        in_maps.append(m)
    return in_maps


def _unshard(results):
    out = np.empty((B, CIN, H, W), np.float32)
    for c in range(N_CORES):
        yc = results[c]["y"].astype(np.float32)
        yv = yc.reshape(128, 4, BL, H, W).transpose(2, 1, 0, 3, 4)
        out[BL * c:BL * (c + 1)] = yv.reshape(BL, CIN, H, W)
    return out


def kernel(**inputs):
    # the fast drained path assumes bs == 0 (true for this problem's
    # setup_inputs); nonzero bs routes every pair through the direct path,
    # which applies bs exactly
    all_direct = bool(np.abs(np.asarray(inputs["bs"])).max() > 0)
    nc = get_program(all_direct)
    in_maps = _host_prep(inputs)
    import os
    trace = bool(os.environ.get("KERNEL_TRACE"))
    kw = {}
    if trace:
        import tempfile
        kw = dict(trace=True, tmpdir=tempfile.mkdtemp(prefix="ktr_"))
        try:
            import ntff_shim  # noqa: F401
        except ImportError:
            pass
    res = bass_utils.run_bass_kernel_spmd(
        nc, in_maps, core_ids=list(range(N_CORES)), **kw)
    if trace and res.exec_time_ns is not None:
        prof = os.environ.get("KERNEL_PROFILE_OUT")
        if prof:
            with open(prof, "w") as f:
                f.write(str(res.exec_time_ns))
        print(f"HW exec time: {res.exec_time_ns} ns")
    return _unshard(res.results)
